# revision 17
# baseline (speedup 1.0000x reference)
"""Causal self-attention (B=8, T=1024, D=2048, H=16) on 8 NeuronCores.

Data-parallel over the batch dim: core i handles batch element i end-to-end
(QKV proj -> causal attention -> out proj). No collectives.

Layout: everything runs on transposed activations. The host feeds x[b].T
([D, T]) in fp16; Q/K are produced d-major ([Dh, T]), V token-major, and the
output projection emits y.T which the host transposes back. All matmul
operands are fp16 (same PE rate as f32r but half the LdWeights/DMA/SBUF
traffic); PSUM accumulation is fp32.

Softmax skips the max-subtraction (scores are ~N(0,1); exp is safely in fp16
range). The denominator is built by summing exp tiles on the DVE and doing a
single ones-column matmul per q-chunk, instead of a PE reduction chain per
k-tile. Causal structure is exploited at q-chunk=256 granularity (k-tiles
0..2jc+1 per chunk), and per-head attention outputs stay resident in SBUF as
the rhs of the output projection (no DRAM round-trip).
"""

import math

import numpy as np

B, T, D = 8, 1024, 2048
H = 16
DH = D // H  # 128
NCT = D // 128  # 16 c-tiles
QC = 256  # q-chunk for causal attention
NQC = T // QC  # 4
SCALE = 1.0 / math.sqrt(DH)
N_CORES = 8

_CACHE = {}


def _build():
    import concourse.bacc as bacc
    import concourse.mybir as mybir
    import concourse.tile as tile

    f32 = mybir.dt.float32
    f16 = mybir.dt.float16
    Exp = mybir.ActivationFunctionType.Exp

    nc = bacc.Bacc(None, target_bir_lowering=False)

    xT = nc.declare_dram_parameter("xT", [D, T], f16, isOutput=False)
    w_qkv = nc.declare_dram_parameter("w_qkv", [D, 3 * D], f16, isOutput=False)
    b_qkv = nc.declare_dram_parameter("b_qkv", [3 * D], f32, isOutput=False)
    b_v = nc.declare_dram_parameter("b_v", [D], f16, isOutput=False)
    w_proj = nc.declare_dram_parameter("w_proj", [D, D], f16, isOutput=False)
    b_proj = nc.declare_dram_parameter("b_proj", [D], f32, isOutput=False)
    outT = nc.declare_dram_parameter("outT", [D, T], f32, isOutput=True)

    with tile.TileContext(nc) as tc:
        with (
            tc.tile_pool(name="xbig", bufs=1) as pool_xbig,
            tc.tile_pool(name="vbig", bufs=1) as pool_vbig,
            tc.tile_pool(name="aobig", bufs=1) as pool_aobig,
            tc.tile_pool(name="qk", bufs=4) as pool_qk,
            tc.tile_pool(name="e", bufs=4) as pool_e,
            tc.tile_pool(name="esum", bufs=2) as pool_esum,
            tc.tile_pool(name="w512", bufs=2) as pool_w512,
            tc.tile_pool(name="wbig", bufs=2) as pool_wbig,
            tc.tile_pool(name="wproj", bufs=2) as pool_wproj,
            tc.tile_pool(name="outp", bufs=2) as pool_out,
            tc.tile_pool(name="den", bufs=3) as pool_den,
            tc.tile_pool(name="misc", bufs=1) as pool_misc,
        ):
            # ---- load x.T resident: 16 tiles [128, 1024], one per c-tile.
            # fc=0 weight DMAs are interleaved ahead of each xT tile and xT
            # is loaded in halves so the first matmuls start early; constant
            # and bias setup is deferred behind the first two c-tiles. ----
            # x streams on the sync DMA queue; weights and biases go on the
            # scalar (ACT) DMA queue so neither stream blocks the other.
            xT_t = []
            w_fc0 = []
            for ct in range(NCT):
                w_t = pool_w512.tile(
                    [128, 512], f16, name="w_fc0", tag="w512", bufs=4
                )
                nc.scalar.dma_start(
                    w_t[:],
                    w_qkv[ct * 128 : (ct + 1) * 128, 2 * D : 2 * D + 512],
                )
                w_fc0.append(w_t)
                t_ = pool_xbig.tile(
                    [128, T], f16, name="xT_t", tag="xbig", bufs=NCT
                )
                for half in range(2):
                    nc.sync.dma_start(
                        t_[:, half * 512 : (half + 1) * 512],
                        xT[
                            ct * 128 : (ct + 1) * 128,
                            half * 512 : (half + 1) * 512,
                        ],
                    )
                xT_t.append(t_)

                if ct == 1:
                    # ---- constants / biases (off the critical DMA path) ----
                    # memset can't target f16 reliably; stage via f32 + copy
                    ones_col_f = pool_misc.tile([128, 1], f32, tag="ones_col_f")
                    nc.vector.memset(ones_col_f[:], 1.0)
                    ones_col = pool_misc.tile([128, 1], f16, tag="ones_col")
                    nc.vector.tensor_copy(ones_col[:], ones_col_f[:])
                    ones_row_f = pool_misc.tile([1, 128], f32, tag="ones_row_f")
                    nc.vector.memset(ones_row_f[:], 1.0)
                    ones_row = pool_misc.tile([1, 128], f16, tag="ones_row")
                    nc.vector.tensor_copy(ones_row[:], ones_row_f[:])

                    # causal masks for the two diagonal k-tiles of a q-chunk:
                    # keep where k <= q  <=>  f - p - 128*r >= 0
                    masks = []
                    mask_f = pool_misc.tile([128, QC], f32, tag="mask_f")
                    for r in range(2):
                        nc.vector.memset(mask_f[:], 1.0)
                        nc.gpsimd.affine_select(
                            out=mask_f[:],
                            in_=mask_f[:],
                            compare_op=mybir.AluOpType.is_ge,
                            fill=0.0,
                            base=-128 * r,
                            pattern=[[1, QC]],
                            channel_multiplier=-1,
                        )
                        m16 = pool_misc.tile([128, QC], f16, name="m16", tag=f"m{r}")
                        nc.vector.tensor_copy(m16[:], mask_f[:])
                        masks.append(m16)

                    # b_qkv as [128, 48]: column j = feature-tile j
                    bqkv_sb = pool_misc.tile([128, 3 * D // 128], f32, tag="bqkv")
                    nc.scalar.dma_start(
                        bqkv_sb[:], b_qkv[:].rearrange("(n p) -> p n", p=128)
                    )
                    bproj_sb = pool_misc.tile([128, D // 128], f32, tag="bproj")
                    nc.scalar.dma_start(
                        bproj_sb[:], b_proj[:].rearrange("(n p) -> p n", p=128)
                    )
                    # V-bias as a [1, 2048] f16 row; added into the V PSUM via
                    # a K=1 ones-matmul (broadcasts across partitions)
                    bv_row = pool_misc.tile([1, D], f16, tag="bv_row")
                    nc.scalar.dma_start(bv_row[:], b_v[:].rearrange("(o f) -> o f", o=1))

            # ---- phase 1: V for all heads, token-major [128, 8, 2048].
            # Token tiles in two halves of 4 so only 4 PSUM banks are held;
            # phase 2's pools grab the other banks without waiting on the
            # phase-1 drain. ----
            V_sb = pool_vbig.tile([128, T // 128, D], f16, tag="vbig")
            with tc.tile_pool(name="p1psum", bufs=4, space="PSUM") as pool_p1:
                for fc in range(D // 512):
                    for th in range(2):
                        if fc == 0 and th == 0:
                            w_ts = w_fc0
                        else:
                            # weights are re-fetched per token half so only 4
                            # w buffers rotate (DMA is idle here anyway)
                            w_ts = []
                            for ct in range(NCT):
                                w_t = pool_w512.tile(
                                    [128, 512], f16, name="w_t", tag="w512", bufs=4
                                )
                                nc.scalar.dma_start(
                                    w_t[:],
                                    w_qkv[
                                        ct * 128 : (ct + 1) * 128,
                                        2 * D + fc * 512 : 2 * D + (fc + 1) * 512,
                                    ],
                                )
                                w_ts.append(w_t)
                        ps_v = [
                            pool_p1.tile([128, 512], f32, name="vps", tag="vps")
                            for _ in range(4)
                        ]
                        for ct in range(NCT):
                            for ti in range(4):
                                tt = th * 4 + ti
                                nc.tensor.matmul(
                                    ps_v[ti][:],
                                    xT_t[ct][:, tt * 128 : (tt + 1) * 128],
                                    w_ts[ct][:],
                                    start=(ct == 0),
                                    stop=False,
                                )
                        for ti in range(4):
                            tt = th * 4 + ti
                            # += b_v (broadcast over tokens), close the group
                            nc.tensor.matmul(
                                ps_v[ti][:],
                                ones_row[:],
                                bv_row[:, fc * 512 : (fc + 1) * 512],
                                start=False,
                                stop=True,
                            )
                            nc.vector.tensor_copy(
                                V_sb[:, tt, fc * 512 : (fc + 1) * 512],
                                ps_v[ti][:],
                            )

            # ---- phase 2: per-head attention, software-pipelined: head
            # h+1's Q/K GEMMs are emitted before head h's attention so the
            # PE keeps working while the DVE finishes bias-adds. w_proj is
            # also staged into SBUF row-contiguous (one c-tile per head) so
            # phase 3 starts with all weights resident. ----
            def emit_qk(h):
                qk = {}
                for s, (base, btile) in (
                    ("q", (0, h)),
                    ("k", (D, NCT + h)),
                ):
                    sb = pool_qk.tile([128, T], f16, name="qk_sb", tag="qk")
                    # strided DMAs for the [D, 128] column block,
                    # c-tile major, split in two for pipelining
                    w_halves = []
                    for hf in range(2):
                        w_t = pool_wbig.tile(
                            [128, NCT // 2, 128],
                            f16,
                            name="w_t",
                            tag="wbig",
                            bufs=5,
                        )
                        nc.sync.dma_start(
                            w_t[:],
                            w_qkv[
                                hf * (D // 2) : (hf + 1) * (D // 2),
                                base + h * 128 : base + (h + 1) * 128,
                            ].rearrange("(n p) f -> p n f", p=128),
                        )
                        w_halves.append(w_t)
                    # ct-outer / jc-inner: consecutive matmuls share the
                    # stationary weight block (one LdWeights per ct); the
                    # two 512-wide chains accumulate in separate banks
                    ps = [
                        pool_qa.tile([128, 512], f32, name="qkps", tag="qa")
                        for _ in range(2)
                    ]
                    for ct in range(NCT):
                        for jc in range(2):
                            nc.tensor.matmul(
                                ps[jc][:],
                                w_halves[ct // 8][:, ct % 8, :],
                                xT_t[ct][:, jc * 512 : (jc + 1) * 512],
                                start=(ct == 0),
                                stop=(ct == NCT - 1),
                            )
                    for jc in range(2):
                        nc.vector.tensor_scalar_add(
                            sb[:, jc * 512 : (jc + 1) * 512],
                            ps[jc][:],
                            bqkv_sb[:, btile : btile + 1],
                        )
                    qk[s] = sb
                return qk

            def emit_attn(h, qk):
                # causal attention, scores transposed [k, q],
                # q-chunks of 256 (k-tiles 0..2jc+1; rest masked)
                ao_t = pool_aobig.tile(
                    [128, T], f16, name="ao_t", tag="aobig", bufs=H
                )
                for jc in range(NQC):
                    nk = 2 * jc + 2
                    ps_y = pool_y.tile([128, QC], f32, tag="y")
                    e_sum = pool_esum.tile([128, QC], f16, tag="esum", bufs=3)
                    for ki in range(nk):
                        ps_s = pool_s.tile([128, QC], f32, tag="mm256")
                        nc.tensor.matmul(
                            ps_s[:],
                            qk["k"][:, ki * 128 : (ki + 1) * 128],
                            qk["q"][:, jc * QC : (jc + 1) * QC],
                            start=True,
                            stop=True,
                        )
                        # exp of the first k-tile lands directly in e_sum
                        e_t = (
                            e_sum
                            if ki == 0
                            else pool_e.tile([128, QC], f16, tag="e", bufs=6)
                        )
                        nc.scalar.activation(e_t[:], ps_s[:], Exp, scale=SCALE)
                        r = ki - 2 * jc
                        if r >= 0:
                            # causal mask for the diagonal k-tiles: one
                            # DVE multiply with a precomputed 0/1 tile
                            nc.vector.tensor_mul(e_t[:], e_t[:], masks[r][:])
                        nc.tensor.matmul(
                            ps_y[:],
                            V_sb[:, ki, h * 128 : (h + 1) * 128],
                            e_t[:],
                            start=(ki == 0),
                            stop=(ki == nk - 1),
                        )
                        if ki > 0:
                            nc.vector.tensor_add(e_sum[:], e_sum[:], e_t[:])
                    # single ones-column matmul closes the denominator
                    ps_d = pool_s.tile([1, QC], f32, name="ps_d", tag="mm256")
                    nc.tensor.matmul(
                        ps_d[:], ones_col[:], e_sum[:], start=True, stop=True
                    )
                    # approx reciprocal (~18 bits; denominators are
                    # bounded away from 0 by the diagonal exp term)
                    inv_d = pool_den.tile([1, QC], f32, tag="invden")
                    nc.vector.reciprocal_approx_fast(out=inv_d[:], in_=ps_d[:])
                    inv16 = pool_den.tile([1, QC], f16, name="inv16", tag="inv16")
                    nc.scalar.copy(inv16[:], inv_d[:])
                    ps_b = pool_s.tile([128, QC], f32, tag="mm256")
                    nc.tensor.matmul(
                        ps_b[:], ones_row[:], inv16[:], start=True, stop=True
                    )
                    # walrus: only one PSUM operand per DVE op -> stage
                    # the broadcast through SBUF
                    bcast_sb = pool_den.tile(
                        [128, QC], f32, name="bcast_sb", tag="bcast"
                    )
                    nc.vector.tensor_copy(bcast_sb[:], ps_b[:])
                    nc.vector.tensor_mul(
                        ao_t[:, jc * QC : (jc + 1) * QC], ps_y[:], bcast_sb[:]
                    )
                return ao_t

            with (
                tc.tile_pool(name="sps", bufs=2, space="PSUM") as pool_s,
                tc.tile_pool(name="qaps", bufs=4, space="PSUM") as pool_qa,
                tc.tile_pool(name="yps", bufs=2, space="PSUM") as pool_y,
            ):
                ao_heads = []
                wp_full = []
                qk_prev = emit_qk(0)
                for h in range(H):
                    # stage one row-contiguous c-tile of w_proj per head on
                    # the scalar DMA queue (ready before phase 3 starts)
                    wp_t = pool_wproj.tile(
                        [128, D], f16, name="wp_t", tag="wproj", bufs=NCT
                    )
                    nc.scalar.dma_start(
                        wp_t[:], w_proj[h * 128 : (h + 1) * 128, :]
                    )
                    wp_full.append(wp_t)

                    if h + 1 < H:
                        qk_next = emit_qk(h + 1)
                    ao_heads.append(emit_attn(h, qk_prev))
                    if h + 1 < H:
                        qk_prev = qk_next

            # ---- phase 3: output projection, emitted transposed.
            # rhs for c-tile ct is exactly head ct's attention output
            # (f = h*128 + dh); weights and activations are all resident. ----
            with tc.tile_pool(name="p3psum", bufs=4, space="PSUM") as pool_p3:
                for dt in range(D // 128):
                    ps3 = [
                        pool_p3.tile([128, 512], f32, name="ps3", tag="mm512")
                        for _ in range(2)
                    ]
                    for ct in range(NCT):
                        for jc in range(2):
                            nc.tensor.matmul(
                                ps3[jc][:],
                                wp_full[ct][:, dt * 128 : (dt + 1) * 128],
                                ao_heads[ct][:, jc * 512 : (jc + 1) * 512],
                                start=(ct == 0),
                                stop=(ct == NCT - 1),
                            )
                    for jc in range(2):
                        o_t = pool_out.tile([128, 512], f32, tag="outp")
                        nc.vector.tensor_scalar_add(
                            o_t[:], ps3[jc][:], bproj_sb[:, dt : dt + 1]
                        )
                        nc.sync.dma_start(
                            outT[dt * 128 : (dt + 1) * 128, jc * 512 : (jc + 1) * 512],
                            o_t[:],
                        )

    nc.compile()
    return nc


def _get_nc():
    if "nc" not in _CACHE:
        _CACHE["nc"] = _build()
    return _CACHE["nc"]


def kernel(x, w_qkv, b_qkv, w_proj, b_proj, _trace=False, _trace_kwargs=None):
    from concourse.bass_utils import run_bass_kernel_spmd

    x = np.asarray(x, dtype=np.float32)
    w_qkv = np.asarray(w_qkv, dtype=np.float32)
    b_qkv = np.asarray(b_qkv, dtype=np.float32)
    w_proj = np.asarray(w_proj, dtype=np.float32)
    b_proj = np.asarray(b_proj, dtype=np.float32)

    w_qkv16 = np.ascontiguousarray(w_qkv.astype(np.float16))
    w_proj16 = np.ascontiguousarray(w_proj.astype(np.float16))
    b_v16 = np.ascontiguousarray(b_qkv[2 * D : 3 * D].astype(np.float16))

    nc = _get_nc()
    in_maps = []
    for i in range(N_CORES):
        in_maps.append(
            {
                "xT": np.ascontiguousarray(x[i].T.astype(np.float16)),
                "w_qkv": w_qkv16,
                "b_qkv": b_qkv,
                "b_v": b_v16,
                "w_proj": w_proj16,
                "b_proj": b_proj,
            }
        )
    res = run_bass_kernel_spmd(
        nc,
        in_maps,
        list(range(N_CORES)),
        trace=_trace,
        **(_trace_kwargs or {}),
    )
    y = np.stack(
        [np.ascontiguousarray(res.results[i]["outT"].T) for i in range(N_CORES)]
    )
    if _trace:
        _CACHE["last_result"] = res
    return y


# revision 19
# speedup vs baseline: 1.1482x; 1.1482x over previous
"""Causal self-attention (B=8, T=1024, D=2048, H=16) on 8 NeuronCores.

Data-parallel over the batch dim: core i handles batch element i end-to-end
(QKV proj -> causal attention -> out proj). No collectives.

Layout: everything runs on transposed activations. The host feeds x[b].T
([D, T]) in fp16; Q/K are produced d-major ([Dh, T]), V token-major, and the
output projection emits y.T which the host transposes back. All matmul
operands are fp16 (same PE rate as f32r but half the LdWeights/DMA/SBUF
traffic); PSUM accumulation is fp32.

Softmax skips the max-subtraction (scores are ~N(0,1); exp is safely in fp16
range). The denominator is built by summing exp tiles on the DVE and doing a
single ones-column matmul per q-chunk, instead of a PE reduction chain per
k-tile. Causal structure is exploited at q-chunk=256 granularity (k-tiles
0..2jc+1 per chunk), and per-head attention outputs stay resident in SBUF as
the rhs of the output projection (no DRAM round-trip).
"""

import math

import numpy as np

B, T, D = 8, 1024, 2048
H = 16
DH = D // H  # 128
NCT = D // 128  # 16 c-tiles
QC = 256  # q-chunk for causal attention
NQC = T // QC  # 4
SCALE = 1.0 / math.sqrt(DH)
N_CORES = 8

_CACHE = {}


def _build():
    import concourse.bacc as bacc
    import concourse.mybir as mybir
    import concourse.tile as tile

    f32 = mybir.dt.float32
    f16 = mybir.dt.float16
    Exp = mybir.ActivationFunctionType.Exp

    nc = bacc.Bacc(None, target_bir_lowering=False)

    xT = nc.declare_dram_parameter("xT", [D, T], f16, isOutput=False)
    w_qkv = nc.declare_dram_parameter("w_qkv", [D, 3 * D], f16, isOutput=False)
    b_qkv = nc.declare_dram_parameter("b_qkv", [3 * D], f32, isOutput=False)
    b_v = nc.declare_dram_parameter("b_v", [D], f16, isOutput=False)
    w_proj = nc.declare_dram_parameter("w_proj", [D, D], f16, isOutput=False)
    b_proj = nc.declare_dram_parameter("b_proj", [D], f32, isOutput=False)
    outT = nc.declare_dram_parameter("outT", [D, T], f32, isOutput=True)

    with tile.TileContext(nc) as tc:
        with (
            tc.tile_pool(name="xbig", bufs=1) as pool_xbig,
            tc.tile_pool(name="vbig", bufs=1) as pool_vbig,
            tc.tile_pool(name="aobig", bufs=1) as pool_aobig,
            tc.tile_pool(name="qk", bufs=4) as pool_qk,
            tc.tile_pool(name="e", bufs=4) as pool_e,
            tc.tile_pool(name="esum", bufs=2) as pool_esum,
            tc.tile_pool(name="w512", bufs=2) as pool_w512,
            tc.tile_pool(name="wbig", bufs=2) as pool_wbig,
            tc.tile_pool(name="wproj", bufs=2) as pool_wproj,
            tc.tile_pool(name="outp", bufs=2) as pool_out,
            tc.tile_pool(name="den", bufs=3) as pool_den,
            tc.tile_pool(name="misc", bufs=1) as pool_misc,
        ):
            # ---- load x.T resident: 16 tiles [128, 1024], one per c-tile.
            # fc=0 weight DMAs are interleaved ahead of each xT tile and xT
            # is loaded in halves so the first matmuls start early; constant
            # and bias setup is deferred behind the first two c-tiles. ----
            # x streams on the sync DMA queue; weights and biases go on the
            # scalar (ACT) DMA queue so neither stream blocks the other.
            xT_t = []
            w_fc0 = []
            for ct in range(NCT):
                w_t = pool_w512.tile(
                    [128, 512], f16, name="w_fc0", tag="w512", bufs=4
                )
                nc.scalar.dma_start(
                    w_t[:],
                    w_qkv[ct * 128 : (ct + 1) * 128, 2 * D : 2 * D + 512],
                )
                w_fc0.append(w_t)
                t_ = pool_xbig.tile(
                    [128, T], f16, name="xT_t", tag="xbig", bufs=NCT
                )
                for half in range(2):
                    nc.sync.dma_start(
                        t_[:, half * 512 : (half + 1) * 512],
                        xT[
                            ct * 128 : (ct + 1) * 128,
                            half * 512 : (half + 1) * 512,
                        ],
                    )
                xT_t.append(t_)

                if ct == 1:
                    # ---- constants / biases (off the critical DMA path) ----
                    # memset can't target f16 reliably; stage via f32 + copy
                    ones_col_f = pool_misc.tile([128, 1], f32, tag="ones_col_f")
                    nc.vector.memset(ones_col_f[:], 1.0)
                    ones_col = pool_misc.tile([128, 1], f16, tag="ones_col")
                    nc.vector.tensor_copy(ones_col[:], ones_col_f[:])
                    ones_row_f = pool_misc.tile([1, 128], f32, tag="ones_row_f")
                    nc.vector.memset(ones_row_f[:], 1.0)
                    ones_row = pool_misc.tile([1, 128], f16, tag="ones_row")
                    nc.vector.tensor_copy(ones_row[:], ones_row_f[:])

                    # causal masks for the two diagonal k-tiles of a q-chunk:
                    # keep where k <= q  <=>  f - p - 128*r >= 0
                    masks = []
                    mask_f = pool_misc.tile([128, QC], f32, tag="mask_f")
                    for r in range(2):
                        nc.vector.memset(mask_f[:], 1.0)
                        nc.gpsimd.affine_select(
                            out=mask_f[:],
                            in_=mask_f[:],
                            compare_op=mybir.AluOpType.is_ge,
                            fill=0.0,
                            base=-128 * r,
                            pattern=[[1, QC]],
                            channel_multiplier=-1,
                        )
                        m16 = pool_misc.tile([128, QC], f16, name="m16", tag=f"m{r}")
                        nc.vector.tensor_copy(m16[:], mask_f[:])
                        masks.append(m16)

                    # b_qkv as [128, 48]: column j = feature-tile j
                    bqkv_sb = pool_misc.tile([128, 3 * D // 128], f32, tag="bqkv")
                    nc.scalar.dma_start(
                        bqkv_sb[:], b_qkv[:].rearrange("(n p) -> p n", p=128)
                    )
                    bproj_sb = pool_misc.tile([128, D // 128], f32, tag="bproj")
                    nc.scalar.dma_start(
                        bproj_sb[:], b_proj[:].rearrange("(n p) -> p n", p=128)
                    )
                    # V-bias as a [1, 2048] f16 row; added into the V PSUM via
                    # a K=1 ones-matmul (broadcasts across partitions)
                    bv_row = pool_misc.tile([1, D], f16, tag="bv_row")
                    nc.scalar.dma_start(bv_row[:], b_v[:].rearrange("(o f) -> o f", o=1))

            # ---- phase 1: V for all heads, token-major [128, 8, 2048] ----
            V_sb = pool_vbig.tile([128, T // 128, D], f16, tag="vbig")
            with tc.tile_pool(name="p1psum", bufs=8, space="PSUM") as pool_p1:
                for fc in range(D // 512):
                    ps_v = [
                        pool_p1.tile([128, 512], f32, name="vps", tag="vps")
                        for _ in range(T // 128)
                    ]
                    for ct in range(NCT):
                        if fc == 0:
                            w_t = w_fc0[ct]
                        else:
                            w_t = pool_w512.tile(
                                [128, 512], f16, name="w_t", tag="w512", bufs=4
                            )
                            nc.scalar.dma_start(
                                w_t[:],
                                w_qkv[
                                    ct * 128 : (ct + 1) * 128,
                                    2 * D + fc * 512 : 2 * D + (fc + 1) * 512,
                                ],
                            )
                        for tt in range(T // 128):
                            nc.tensor.matmul(
                                ps_v[tt][:],
                                xT_t[ct][:, tt * 128 : (tt + 1) * 128],
                                w_t[:],
                                start=(ct == 0),
                                stop=False,
                            )
                    for tt in range(T // 128):
                        # += b_v (broadcast over tokens), close the group
                        nc.tensor.matmul(
                            ps_v[tt][:],
                            ones_row[:],
                            bv_row[:, fc * 512 : (fc + 1) * 512],
                            start=False,
                            stop=True,
                        )
                        nc.vector.tensor_copy(
                            V_sb[:, tt, fc * 512 : (fc + 1) * 512],
                            ps_v[tt][:],
                        )

            # ---- phase 2: per-head attention, software-pipelined: head
            # h+1's Q/K GEMMs are emitted before head h's attention so the
            # PE keeps working while the DVE finishes bias-adds. w_proj is
            # also staged into SBUF row-contiguous (one c-tile per head) so
            # phase 3 starts with all weights resident. ----
            def emit_qk(h):
                qk = {}
                for s, (base, btile) in (
                    ("q", (0, h)),
                    ("k", (D, NCT + h)),
                ):
                    sb = pool_qk.tile([128, T], f16, name="qk_sb", tag="qk")
                    # strided DMAs for the [D, 128] column block,
                    # c-tile major, split in two for pipelining
                    w_halves = []
                    for hf in range(2):
                        w_t = pool_wbig.tile(
                            [128, NCT // 2, 128],
                            f16,
                            name="w_t",
                            tag="wbig",
                            bufs=5,
                        )
                        nc.sync.dma_start(
                            w_t[:],
                            w_qkv[
                                hf * (D // 2) : (hf + 1) * (D // 2),
                                base + h * 128 : base + (h + 1) * 128,
                            ].rearrange("(n p) f -> p n f", p=128),
                        )
                        w_halves.append(w_t)
                    # ct-outer / jc-inner: consecutive matmuls share the
                    # stationary weight block (one LdWeights per ct); the
                    # two 512-wide chains accumulate in separate banks
                    ps = [
                        pool_qa.tile([128, 512], f32, name="qkps", tag="qa")
                        for _ in range(2)
                    ]
                    for ct in range(NCT):
                        for jc in range(2):
                            nc.tensor.matmul(
                                ps[jc][:],
                                w_halves[ct // 8][:, ct % 8, :],
                                xT_t[ct][:, jc * 512 : (jc + 1) * 512],
                                start=(ct == 0),
                                stop=(ct == NCT - 1),
                            )
                    for jc in range(2):
                        nc.vector.tensor_scalar_add(
                            sb[:, jc * 512 : (jc + 1) * 512],
                            ps[jc][:],
                            bqkv_sb[:, btile : btile + 1],
                        )
                    qk[s] = sb
                return qk

            def emit_attn(h, qk):
                # causal attention, scores transposed [k, q],
                # q-chunks of 256 (k-tiles 0..2jc+1; rest masked)
                ao_t = pool_aobig.tile(
                    [128, T], f16, name="ao_t", tag="aobig", bufs=H
                )
                for jc in range(NQC):
                    nk = 2 * jc + 2
                    ps_y = pool_y.tile([128, QC], f32, tag="y")
                    e_sum = pool_esum.tile([128, QC], f16, tag="esum", bufs=3)
                    for ki in range(nk):
                        ps_s = pool_s.tile([128, QC], f32, tag="mm256")
                        nc.tensor.matmul(
                            ps_s[:],
                            qk["k"][:, ki * 128 : (ki + 1) * 128],
                            qk["q"][:, jc * QC : (jc + 1) * QC],
                            start=True,
                            stop=True,
                        )
                        # exp of the first k-tile lands directly in e_sum
                        e_t = (
                            e_sum
                            if ki == 0
                            else pool_e.tile([128, QC], f16, tag="e", bufs=6)
                        )
                        nc.scalar.activation(e_t[:], ps_s[:], Exp, scale=SCALE)
                        r = ki - 2 * jc
                        if r >= 0:
                            # causal mask for the diagonal k-tiles: one
                            # DVE multiply with a precomputed 0/1 tile
                            nc.vector.tensor_mul(e_t[:], e_t[:], masks[r][:])
                        nc.tensor.matmul(
                            ps_y[:],
                            V_sb[:, ki, h * 128 : (h + 1) * 128],
                            e_t[:],
                            start=(ki == 0),
                            stop=(ki == nk - 1),
                        )
                        if ki > 0:
                            nc.vector.tensor_add(e_sum[:], e_sum[:], e_t[:])
                    # single ones-column matmul closes the denominator
                    ps_d = pool_s.tile([1, QC], f32, name="ps_d", tag="mm256")
                    nc.tensor.matmul(
                        ps_d[:], ones_col[:], e_sum[:], start=True, stop=True
                    )
                    # approx reciprocal (~18 bits; denominators are
                    # bounded away from 0 by the diagonal exp term)
                    inv_d = pool_den.tile([1, QC], f32, tag="invden")
                    nc.vector.reciprocal_approx_fast(out=inv_d[:], in_=ps_d[:])
                    inv16 = pool_den.tile([1, QC], f16, name="inv16", tag="inv16")
                    nc.scalar.copy(inv16[:], inv_d[:])
                    ps_b = pool_s.tile([128, QC], f32, tag="mm256")
                    nc.tensor.matmul(
                        ps_b[:], ones_row[:], inv16[:], start=True, stop=True
                    )
                    # walrus: only one PSUM operand per DVE op -> stage
                    # the broadcast through SBUF
                    bcast_sb = pool_den.tile(
                        [128, QC], f32, name="bcast_sb", tag="bcast"
                    )
                    nc.vector.tensor_copy(bcast_sb[:], ps_b[:])
                    nc.vector.tensor_mul(
                        ao_t[:, jc * QC : (jc + 1) * QC], ps_y[:], bcast_sb[:]
                    )
                return ao_t

            with (
                tc.tile_pool(name="sps", bufs=4, space="PSUM") as pool_s,
                tc.tile_pool(name="qaps", bufs=2, space="PSUM") as pool_qa,
                tc.tile_pool(name="yps", bufs=2, space="PSUM") as pool_y,
            ):
                ao_heads = []
                wp_full = []
                for h in range(H):
                    # stage one row-contiguous c-tile of w_proj per head on
                    # the scalar DMA queue (ready before phase 3 starts)
                    wp_t = pool_wproj.tile(
                        [128, D], f16, name="wp_t", tag="wproj", bufs=NCT
                    )
                    nc.scalar.dma_start(
                        wp_t[:], w_proj[h * 128 : (h + 1) * 128, :]
                    )
                    wp_full.append(wp_t)

                    qk = emit_qk(h)
                    ao_heads.append(emit_attn(h, qk))

            # ---- phase 3: output projection, emitted transposed.
            # rhs for c-tile ct is exactly head ct's attention output
            # (f = h*128 + dh); weights and activations are all resident. ----
            with tc.tile_pool(name="p3psum", bufs=4, space="PSUM") as pool_p3:
                for dt in range(D // 128):
                    ps3 = [
                        pool_p3.tile([128, 512], f32, name="ps3", tag="mm512")
                        for _ in range(2)
                    ]
                    for ct in range(NCT):
                        for jc in range(2):
                            nc.tensor.matmul(
                                ps3[jc][:],
                                wp_full[ct][:, dt * 128 : (dt + 1) * 128],
                                ao_heads[ct][:, jc * 512 : (jc + 1) * 512],
                                start=(ct == 0),
                                stop=(ct == NCT - 1),
                            )
                    for jc in range(2):
                        o_t = pool_out.tile([128, 512], f32, tag="outp")
                        nc.vector.tensor_scalar_add(
                            o_t[:], ps3[jc][:], bproj_sb[:, dt : dt + 1]
                        )
                        nc.sync.dma_start(
                            outT[dt * 128 : (dt + 1) * 128, jc * 512 : (jc + 1) * 512],
                            o_t[:],
                        )

    nc.compile()
    return nc


def _get_nc():
    if "nc" not in _CACHE:
        _CACHE["nc"] = _build()
    return _CACHE["nc"]


def kernel(x, w_qkv, b_qkv, w_proj, b_proj, _trace=False, _trace_kwargs=None):
    from concourse.bass_utils import run_bass_kernel_spmd

    x = np.asarray(x, dtype=np.float32)
    w_qkv = np.asarray(w_qkv, dtype=np.float32)
    b_qkv = np.asarray(b_qkv, dtype=np.float32)
    w_proj = np.asarray(w_proj, dtype=np.float32)
    b_proj = np.asarray(b_proj, dtype=np.float32)

    w_qkv16 = np.ascontiguousarray(w_qkv.astype(np.float16))
    w_proj16 = np.ascontiguousarray(w_proj.astype(np.float16))
    b_v16 = np.ascontiguousarray(b_qkv[2 * D : 3 * D].astype(np.float16))

    nc = _get_nc()
    in_maps = []
    for i in range(N_CORES):
        in_maps.append(
            {
                "xT": np.ascontiguousarray(x[i].T.astype(np.float16)),
                "w_qkv": w_qkv16,
                "b_qkv": b_qkv,
                "b_v": b_v16,
                "w_proj": w_proj16,
                "b_proj": b_proj,
            }
        )
    res = run_bass_kernel_spmd(
        nc,
        in_maps,
        list(range(N_CORES)),
        trace=_trace,
        **(_trace_kwargs or {}),
    )
    y = np.stack(
        [np.ascontiguousarray(res.results[i]["outT"].T) for i in range(N_CORES)]
    )
    if _trace:
        _CACHE["last_result"] = res
    return y
